# revision 1
# baseline (speedup 1.0000x reference)
"""Bass/Trainium2 kernel for a 3-layer bidirectional GCN (DGCN).

Math per layer (matches PyG GCNConv with self-loops + symmetric norm):
  hf = Ahat_f @ (h @ Wf) + bf      (messages src->dst)
  hb = Ahat_b @ (h @ Wb) + bb      (messages dst->src)
  h  = relu(concat(hf, hb))        (no relu on last layer)

Device strategy (8 NeuronCores, SPMD):
  - nodes padded to N'=8*NPC, core c owns node rows [c*NPC, (c+1)*NPC)
  - edges partitioned by destination core; per-edge metadata (gather index +
    dest slot) is host-precomputed graph preprocessing
  - symmetric norm folded structurally: hs = (h @ W) * dinv[src] at the
    transform epilogue (ACT per-partition scale), out = dinv[dst] * sum + b
    at the aggregation epilogue; the selector matrix is a pure one-hot
  - transform: h kept channel-major [256, NPC] so h tiles are direct matmul
    lhsT operands; produces the node-major gather table hs
  - layer 0: full x replicated on every core -> no communication
  - layers 1,2: sharded transform + AllGather of hs (4 quarter sub-AGs per
    direction so the next layer can start as quarters arrive)
  - aggregation: dma_gather 128-edge chunks from hs (node-major rows), build
    the one-hot S01[e, b, v] = (iota[v]==dstloc[e,b]) for a whole gather call
    in ONE batched DVE tensor_tensor (stride-0 broadcast APs), then
    matmul-accumulate on PE: PSUM[ch,v] += G[e,ch]^T @ S[e,v].  Per
    dest-block of 128 nodes there are exactly KW chunks per source-window
    (host pads edge groups), so the SPMD program is identical on every core.
  - final layer runs the matmul the other way round (PSUM[v,ch] += S^T @ G)
    to emit node-major fp32 output directly.
"""

import numpy as np

P = 128          # partition size / dest-block size / feature width
N_CORES = 8


def _bf16():
    import ml_dtypes
    return ml_dtypes.bfloat16


class Cfg:
    def __init__(self, n_nodes, hid=128, npc=12800, sb=6, feat="bf16"):
        assert npc % 512 == 0
        self.n_nodes = n_nodes
        self.hid = hid
        self.npc = npc                      # nodes per core (padded)
        self.np_total = npc * N_CORES
        assert self.np_total >= n_nodes
        self.qsz = npc // 4                 # rows per quarter (per core)
        self.nbq = self.qsz // P            # blocks per quarter
        self.bpc = npc // P                 # blocks per core
        self.agrows = 8 * self.qsz          # rows of each AG window buffer
        assert self.agrows <= 32767         # int16 gather indices
        self.sb = sb                        # superblock: live PSUM tiles
        self.feat = feat                    # "bf16" or "f32"

    @property
    def runs(self):
        return [list(range(i, min(i + self.sb, self.bpc)))
                for i in range(0, self.bpc, self.sb)]


# ----------------------------------------------------------------------------
# host-side graph preprocessing
# ----------------------------------------------------------------------------

def _group_edges(s, d, cfg):
    """Sort one core's incoming edges by (dest block, source window)."""
    npc, qsz = cfg.npc, cfg.qsz
    lb = (d % npc) // P
    qs = (s % npc) // qsz
    key = lb * 4 + qs
    order = np.argsort(key, kind="stable")
    return order, key[order]


def _scan_kw(s, d, cfg):
    kw = 1
    core_of = d // cfg.npc
    for c in range(N_CORES):
        m = core_of == c
        _, key = _group_edges(s[m], d[m], cfg)
        counts = np.bincount(key, minlength=cfg.bpc * 4)
        kw = max(kw, int(np.ceil(counts.max() / P)))
    return kw


def _build_direction(s, d, cfg, kw, dt_np):
    """Per-core padded edge metadata for one message direction.

    Per core:
      idx [128, idx_cols] int16 : gather row indices, per-call 16-row wrapped
                                  and replicated across the 8 Q7 cores
      dst [128, nchunks] feat   : dest slot within block (-1 for pad edges)
    Order: for run in superblock-runs: for q in windows: for b in run: chunks.
    """
    npc, qsz, bpc = cfg.npc, cfg.qsz, cfg.bpc
    core_of = d // npc
    slots = kw * P
    out = []
    for c in range(N_CORES):
        m = core_of == c
        es, ed = s[m], d[m]
        order, key = _group_edges(es, ed, cfg)
        es, ed = es[order], ed[order]
        counts = np.bincount(key, minlength=bpc * 4)
        assert counts.max() <= slots
        starts = np.zeros(bpc * 4, np.int64)
        starts[1:] = np.cumsum(counts)[:-1]
        rank = np.arange(len(key)) - starts[key]
        flatpos = key.astype(np.int64) * slots + rank
        idx_pad = np.zeros(bpc * 4 * slots, np.int16)
        dst_pad = np.full(bpc * 4 * slots, -1.0, np.float32)
        idx_pad[flatpos] = ((es // npc) * qsz + (es % npc) % qsz).astype(np.int16)
        dst_pad[flatpos] = (ed % P).astype(np.float32)
        idx_pad = idx_pad.reshape(bpc, 4, kw, P)
        dst_pad = dst_pad.reshape(bpc, 4, kw, P)
        idx_cols, dst_cols = [], []
        for run in cfg.runs:
            for q in range(4):
                flat = idx_pad[run[0]:run[-1] + 1, q].reshape(-1)
                idx_cols.append(np.tile(flat.reshape(-1, 16).T, (8, 1)))
                dst_cols.append(dst_pad[run[0]:run[-1] + 1, q].reshape(-1, P).T)
        out.append(dict(
            idx=np.ascontiguousarray(np.concatenate(idx_cols, axis=1)),
            dst=np.ascontiguousarray(
                np.concatenate(dst_cols, axis=1)).astype(dt_np),
        ))
    return out


def host_prep(inputs, cfg):
    x = np.asarray(inputs["x"], np.float32)
    ei = np.asarray(inputs["edge_index"], np.int64)
    src, dst = ei[0], ei[1]
    n = cfg.n_nodes
    sl = np.arange(n, dtype=np.int64)
    pad1 = np.concatenate([np.ones(n), np.zeros(cfg.np_total - n)])

    deg_f = np.bincount(dst, minlength=cfg.np_total) + pad1
    deg_b = np.bincount(src, minlength=cfg.np_total) + pad1
    with np.errstate(divide="ignore"):
        dinv_f = np.where(deg_f > 0, 1.0 / np.sqrt(deg_f), 0.0)
        dinv_f = dinv_f.astype(np.float32)
        dinv_b = np.where(deg_b > 0, 1.0 / np.sqrt(deg_b), 0.0)
        dinv_b = dinv_b.astype(np.float32)

    dt_np = np.float32 if cfg.feat == "f32" else _bf16()

    sf = np.concatenate([src, sl])
    df = np.concatenate([dst, sl])
    kw = max(_scan_kw(sf, df, cfg), _scan_kw(df, sf, cfg))
    meta_f = _build_direction(sf, df, cfg, kw, dt_np)
    meta_b = _build_direction(df, sf, cfg, kw, dt_np)

    xp = np.zeros((cfg.np_total, cfg.hid), np.float32)
    xp[:n] = x
    x_cm = np.ascontiguousarray(xp.T).astype(dt_np)

    wslots = []
    for lname in ("0", "1", "2"):
        for dname in ("f", "b"):
            W = np.asarray(inputs[f"W{dname}{lname}"], np.float32)
            if W.shape[0] == cfg.hid:
                wslots.append(W)
            else:
                wslots.append(W[:cfg.hid])
                wslots.append(W[cfg.hid:])
    w_all = np.concatenate(wslots, axis=1).astype(dt_np)

    bias_all = np.zeros((128, 6), np.float32)
    for li, lname in enumerate(("0", "1", "2")):
        for di, dname in enumerate(("f", "b")):
            bias_all[:, li * 2 + di] = np.asarray(
                inputs[f"b{dname}{lname}"], np.float32)
    bias2 = np.zeros((128, 256), np.float32)
    bias2[:, 0:128] = np.asarray(inputs["bf2"], np.float32)[None, :]
    bias2[:, 128:256] = np.asarray(inputs["bb2"], np.float32)[None, :]

    iota_t = np.tile(np.arange(P, dtype=np.float32)[None, :],
                     (P, 1)).astype(dt_np)

    # per-partition dinv columns: col g holds dinv[g*128:(g+1)*128]
    dinv_cols = {
        "f": np.ascontiguousarray(
            dinv_f.reshape(8 * cfg.bpc, P).T),       # [128, 8*bpc] f32
        "b": np.ascontiguousarray(
            dinv_b.reshape(8 * cfg.bpc, P).T),
    }
    in_maps = []
    for c in range(N_CORES):
        # broadcast dinv over partitions for the free-dim epilogue scaling
        dbc = {
            dn: np.ascontiguousarray(np.tile(
                dv[c * cfg.npc:(c + 1) * cfg.npc][None, :],
                (P, 1))).astype(dt_np)
            for dn, dv in (("f", dinv_f), ("b", dinv_b))
        }
        in_maps.append(dict(
            x_cm=x_cm,
            w_all=w_all,
            bias_all=bias_all,
            bias2=bias2,
            iota_t=iota_t,
            dinv_all_f=dinv_cols["f"],
            dinv_all_b=dinv_cols["b"],
            dinv_loc_f=np.ascontiguousarray(
                dinv_f[c * cfg.npc:(c + 1) * cfg.npc].reshape(cfg.bpc, P).T),
            dinv_loc_b=np.ascontiguousarray(
                dinv_b[c * cfg.npc:(c + 1) * cfg.npc].reshape(cfg.bpc, P).T),
            dinv_bc_f=dbc["f"],
            dinv_bc_b=dbc["b"],
            meta_idx_f=meta_f[c]["idx"],
            meta_dst_f=meta_f[c]["dst"],
            meta_idx_b=meta_b[c]["idx"],
            meta_dst_b=meta_b[c]["dst"],
        ))
    return in_maps, kw


# ----------------------------------------------------------------------------
# device program
# ----------------------------------------------------------------------------

def build_program(cfg, kw):
    import bass_rust
    import concourse.bass as bass
    import concourse.bacc as bacc
    import concourse.tile as tile
    import concourse.mybir as mybir
    from concourse._compat import get_trn_type
    from contextlib import ExitStack

    dt = mybir.dt
    dt_feat = dt.float32 if cfg.feat == "f32" else dt.bfloat16
    AF = mybir.ActivationFunctionType
    ALU = mybir.AluOpType

    npc, qsz, nbq, bpc, sb = cfg.npc, cfg.qsz, cfg.nbq, cfg.bpc, cfg.sb
    runs = cfg.runs
    nchunks = bpc * 4 * kw
    idx_cols_total = nchunks * P // 16

    def bcast_last(ap, n):
        return bass_rust.AP(ap.tensor, ap.offset, list(ap.ap) + [[0, n]])

    def bcast_mid(ap, n):
        a = list(ap.ap)
        return bass_rust.AP(ap.tensor, ap.offset, [a[0], [0, n], a[1]])

    nc = bacc.Bacc(get_trn_type() or "TRN2", target_bir_lowering=False,
                   debug=False)

    x_cm = nc.dram_tensor("x_cm", [P, cfg.np_total], dt_feat,
                          kind="ExternalInput")
    w_all = nc.dram_tensor("w_all", [P, 10 * P], dt_feat, kind="ExternalInput")
    bias_all = nc.dram_tensor("bias_all", [P, 6], dt.float32,
                              kind="ExternalInput")
    bias2 = nc.dram_tensor("bias2", [P, 256], dt.float32, kind="ExternalInput")
    iota_in = nc.dram_tensor("iota_t", [P, P], dt_feat, kind="ExternalInput")
    dinv_all = {d: nc.dram_tensor(f"dinv_all_{d}", [P, 8 * bpc], dt.float32,
                                  kind="ExternalInput") for d in ("f", "b")}
    dinv_bc = {d: nc.dram_tensor(f"dinv_bc_{d}", [P, npc], dt_feat,
                                 kind="ExternalInput") for d in ("f", "b")}
    meta = {}
    for dname in ("f", "b"):
        meta[dname] = dict(
            idx=nc.dram_tensor(f"meta_idx_{dname}", [128, idx_cols_total],
                               dt.int16, kind="ExternalInput"),
            dst=nc.dram_tensor(f"meta_dst_{dname}", [P, nchunks], dt_feat,
                               kind="ExternalInput"),
        )
    out_t = nc.dram_tensor("out", [npc, 2 * P], dt.float32,
                           kind="ExternalOutput")

    agbuf = {d: [nc.dram_tensor(f"ag_{d}{q}", [cfg.agrows, P], dt_feat,
                                addr_space="Shared")
                 for q in range(4)] for d in ("f", "b")}
    hs_loc = {d: [nc.dram_tensor(f"hs_{d}{q}", [qsz, P], dt_feat)
                  for q in range(4)] for d in ("f", "b")}
    h_q = [nc.dram_tensor(f"h_{q}", [2 * P, qsz], dt_feat) for q in range(4)]

    wslot = {}
    col = 0
    for li in range(3):
        for dname in ("f", "b"):
            for h in range(1 if li == 0 else 2):
                wslot[(li, dname, h)] = col
                col += P

    with tile.TileContext(nc) as tc, ExitStack() as ctx:
        cpool = ctx.enter_context(tc.tile_pool(name="consts", bufs=1))
        xhpool = ctx.enter_context(tc.tile_pool(name="xh", bufs=2))
        gpool = ctx.enter_context(tc.tile_pool(name="gather", bufs=3))
        ipool = ctx.enter_context(tc.tile_pool(name="idx", bufs=3))
        mpool = ctx.enter_context(tc.tile_pool(name="meta", bufs=4))
        spool = ctx.enter_context(tc.tile_pool(name="sel", bufs=3))
        dpool = ctx.enter_context(tc.tile_pool(name="dinvq", bufs=3))
        tpool = ctx.enter_context(tc.tile_pool(name="tmp", bufs=4))
        stpool = ctx.enter_context(tc.tile_pool(name="stage", bufs=3))
        st2pool = ctx.enter_context(tc.tile_pool(name="stage2", bufs=2))
        pa = ctx.enter_context(tc.tile_pool(name="psum_a", bufs=sb,
                                            space="PSUM"))
        pt = ctx.enter_context(tc.tile_pool(name="psum_t", bufs=2,
                                            space="PSUM"))

        iota_s = cpool.tile([P, P], dt_feat, tag="iota")
        nc.sync.dma_start(out=iota_s[:, :], in_=iota_in[:, :])
        w_s = cpool.tile([P, 10 * P], dt_feat, tag="wall")
        nc.sync.dma_start(out=w_s[:, :], in_=w_all[:, :])
        bias_s = cpool.tile([P, 6], dt.float32, tag="bias")
        nc.sync.dma_start(out=bias_s[:, :], in_=bias_all[:, :])
        bias2_s = cpool.tile([P, 256], dt.float32, tag="bias2")
        nc.sync.dma_start(out=bias2_s[:, :], in_=bias2[:, :])
        dinv_all_s = {}
        for dname in ("f", "b"):
            t = cpool.tile([P, 8 * bpc], dt.float32, tag=f"dinva_{dname}",
                           name=f"dinva_{dname}")
            nc.sync.dma_start(out=t[:, :], in_=dinv_all[dname][:, :])
            dinv_all_s[dname] = t

        def wap(li, dname, h):
            c0 = wslot[(li, dname, h)]
            return w_s[:, c0:c0 + P]

        def dinv_col(dname, g):
            """per-partition dinv for global block g (f32 [128,1])."""
            return dinv_all_s[dname][:, g:g + 1]

        # ---- layer 0 transform: full x -> agbuf directly -------------------
        def l0_transform():
            for q in range(4):
                for r in range(N_CORES):
                    c0 = r * npc + q * qsz
                    xt = xhpool.tile([P, qsz], dt_feat, tag="ht")
                    nc.sync.dma_start(out=xt[:, :], in_=x_cm[:, c0:c0 + qsz])
                    for dname in ("f", "b"):
                        stg = stpool.tile([P, qsz], dt_feat, tag="stg")
                        for j in range(nbq):
                            g = r * bpc + q * nbq + j
                            ps = pt.tile([P, P], dt.float32, tag="pt")
                            nc.tensor.matmul(ps[:, :],
                                             lhsT=xt[:, j * P:(j + 1) * P],
                                             rhs=wap(0, dname, 0),
                                             start=True, stop=True)
                            nc.scalar.activation(stg[:, j * P:(j + 1) * P],
                                                 ps[:, :], AF.Copy,
                                                 scale=dinv_col(dname, g))
                        nc.sync.dma_start(
                            out=agbuf[dname][q][r * qsz:(r + 1) * qsz, :]
                            .rearrange("(j p) c -> p j c", p=P),
                            in_=stg[:, :].rearrange("p (j c) -> p j c", c=P))

        def aggregate(li, core_rank_unused=None):
            last = li == 2
            for dname in ("f", "b"):
                di = 0 if dname == "f" else 1
                md = meta[dname]
                chunk0 = 0
                icol0 = 0
                stg_q = {}
                dinvq = {}
                blocks_done = {q: 0 for q in range(4)}
                for run in runs:
                    nb = len(run)
                    psums = {b: pa.tile([P, P], dt.float32, tag="pa",
                                        name=f"pa_{li}{dname}{b}")
                             for b in run}
                    for q in range(4):
                        L = nb * kw * P
                        icols = L // 16
                        it = ipool.tile([128, sb * kw * P // 16], dt.int16,
                                        tag="idx")
                        nc.sync.dma_start(
                            out=it[:, 0:icols],
                            in_=md["idx"][:, icol0:icol0 + icols])
                        gt = gpool.tile([P, sb * kw, P], dt_feat, tag="g")
                        nc.gpsimd.dma_gather(
                            out_ap=gt[:, 0:nb * kw, :],
                            in_ap=agbuf[dname][q][:, :],
                            idxs_ap=it[:, 0:icols],
                            num_idxs=L,
                            num_idxs_reg=L,
                            elem_size=P,
                            single_packet=False,
                        )
                        dtile = mpool.tile([P, sb * kw], dt_feat, tag="dst")
                        nc.sync.dma_start(
                            out=dtile[:, 0:nb * kw],
                            in_=md["dst"][:, chunk0:chunk0 + nb * kw])
                        st = spool.tile([P, sb * kw, P], dt_feat, tag="sel")
                        nc.vector.tensor_tensor(
                            st[:, 0:nb * kw, :],
                            bcast_mid(iota_s[:, :], nb * kw),
                            bcast_last(dtile[:, 0:nb * kw], P),
                            ALU.is_equal)
                        for bi, b in enumerate(run):
                            for k in range(kw):
                                cc = bi * kw + k
                                first = (q == 0 and k == 0)
                                lastmm = (q == 3 and k == kw - 1)
                                if last:
                                    nc.tensor.matmul(
                                        psums[b][:, :], lhsT=st[:, cc, :],
                                        rhs=gt[:, cc, :],
                                        start=first, stop=lastmm)
                                else:
                                    nc.tensor.matmul(
                                        psums[b][:, :], lhsT=gt[:, cc, :],
                                        rhs=st[:, cc, :],
                                        start=first, stop=lastmm)
                        chunk0 += nb * kw
                        icol0 += icols
                    for b in run:
                        qb = b // nbq
                        j = b % nbq
                        if qb not in stg_q:
                            if last:
                                stg_q[qb] = st2pool.tile(
                                    [P, qsz], dt.float32, tag="st2",
                                    name=f"st2_{dname}{qb}")
                            else:
                                stg_q[qb] = stpool.tile(
                                    [P, qsz], dt_feat, tag="stg",
                                    name=f"stgh_{li}{dname}{qb}")
                        if last:
                            # out[v,ch] = dinv_d[v]*psum + bias2[ch]
                            nc.vector.scalar_tensor_tensor(
                                out=stg_q[qb][:, j * P:(j + 1) * P],
                                in0=psums[b][:, :],
                                scalar=dinv_loc_s[dname][:, b:b + 1],
                                in1=bias2_s[:, di * P:(di + 1) * P],
                                op0=ALU.mult, op1=ALU.add)
                        else:
                            if qb not in dinvq:
                                dq = dpool.tile([P, qsz], dt_feat,
                                                tag="dinvq",
                                                name=f"dq_{li}{dname}{qb}")
                                nc.sync.dma_start(
                                    out=dq[:, :],
                                    in_=dinv_bc[dname][:, qb * qsz:
                                                       (qb + 1) * qsz])
                                dinvq[qb] = dq
                            tmp = tpool.tile([P, P], dt.float32, tag="tmp")
                            nc.vector.tensor_tensor(
                                tmp[:, :], psums[b][:, :],
                                dinvq[qb][:, j * P:(j + 1) * P], ALU.mult)
                            bcol = li * 2 + di
                            nc.scalar.activation(
                                stg_q[qb][:, j * P:(j + 1) * P],
                                tmp[:, :], AF.Relu,
                                bias=bias_s[:, bcol:bcol + 1])
                        blocks_done[qb] += 1
                        if blocks_done[qb] == nbq:
                            if last:
                                nc.sync.dma_start(
                                    out=out_t[qb * qsz:(qb + 1) * qsz,
                                              di * P:(di + 1) * P]
                                    .rearrange("(j p) c -> p j c", p=P),
                                    in_=stg_q[qb][:, :]
                                    .rearrange("p (j c) -> p j c", c=P))
                            else:
                                nc.sync.dma_start(
                                    out=h_q[qb][di * P:(di + 1) * P, :],
                                    in_=stg_q[qb][:, :])
                            del stg_q[qb]
                            dinvq.pop(qb, None)

        def transform(li, core_rank):
            for dname in ("f", "b"):
                for q in range(4):
                    h0 = xhpool.tile([P, qsz], dt_feat, tag="ht")
                    h1 = xhpool.tile([P, qsz], dt_feat, tag="ht")
                    nc.sync.dma_start(out=h0[:, :], in_=h_q[q][0:P, :])
                    nc.sync.dma_start(out=h1[:, :], in_=h_q[q][P:2 * P, :])
                    stg = stpool.tile([P, qsz], dt_feat, tag="stg")
                    for j in range(nbq):
                        ps = pt.tile([P, P], dt.float32, tag="pt")
                        nc.tensor.matmul(ps[:, :],
                                         lhsT=h0[:, j * P:(j + 1) * P],
                                         rhs=wap(li, dname, 0),
                                         start=True, stop=False)
                        nc.tensor.matmul(ps[:, :],
                                         lhsT=h1[:, j * P:(j + 1) * P],
                                         rhs=wap(li, dname, 1),
                                         start=False, stop=True)
                        nc.scalar.activation(
                            stg[:, j * P:(j + 1) * P], ps[:, :], AF.Copy,
                            scale=dinv_loc_col(dname, q * nbq + j))
                    nc.sync.dma_start(
                        out=hs_loc[dname][q][:, :]
                        .rearrange("(j p) c -> p j c", p=P),
                        in_=stg[:, :].rearrange("p (j c) -> p j c", c=P))
                    nc.gpsimd.collective_compute(
                        "AllGather",
                        mybir.AluOpType.bypass,
                        replica_groups=[list(range(N_CORES))],
                        ins=[hs_loc[dname][q][:, :].opt()],
                        outs=[agbuf[dname][q][:, :].opt()],
                    )

        def dinv_loc_col(dname, lb):
            # local block lb of this core = global block rank*bpc+lb; but the
            # SPMD program is shared, so index via the per-core broadcast
            # input instead: dinv_bc[dname][:, lb*P:(lb+1)*P] column j holds
            # dinv of node lb*P+j, identical down the partition dim.  For a
            # per-partition [128,1] scale we need dinv values ALONG
            # partitions, which dinv_bc cannot provide; use dinv_loc below.
            return dinv_loc_s[dname][:, lb:lb + 1]

        # per-core local per-partition dinv columns: [128, bpc]
        dinv_loc = {d: nc.dram_tensor(f"dinv_loc_{d}", [P, bpc], dt.float32,
                                      kind="ExternalInput")
                    for d in ("f", "b")}
        dinv_loc_s = {}
        for dname in ("f", "b"):
            t = cpool.tile([P, bpc], dt.float32, tag=f"dinvl_{dname}",
                           name=f"dinvl_{dname}")
            nc.sync.dma_start(out=t[:, :], in_=dinv_loc[dname][:, :])
            dinv_loc_s[dname] = t

        with nc.named_scope("L0T"):
            l0_transform()
        with nc.named_scope("agg0"):
            aggregate(0)
        with nc.named_scope("T1"):
            transform(1, None)
        with nc.named_scope("agg1"):
            aggregate(1)
        with nc.named_scope("T2"):
            transform(2, None)
        with nc.named_scope("agg2"):
            aggregate(2)

    nc.compile()
    return nc


# ----------------------------------------------------------------------------
# entry point
# ----------------------------------------------------------------------------

def run(inputs, cfg, trace=False):
    from concourse.bass_utils import run_bass_kernel_spmd
    in_maps, kw = host_prep(inputs, cfg)
    nc = build_program(cfg, kw)
    res = run_bass_kernel_spmd(nc, in_maps, list(range(N_CORES)), trace=trace)
    outs = [res.results[c]["out"] for c in range(N_CORES)]
    full = np.concatenate(outs, axis=0)[:cfg.n_nodes]
    return np.asarray(full, np.float32), res


def kernel(**inputs):
    cfg = Cfg(n_nodes=inputs["x"].shape[0])
    out, _ = run(inputs, cfg)
    return out



# revision 4
# speedup vs baseline: 1.4670x; 1.4670x over previous
"""Bass/Trainium2 kernel for a 3-layer bidirectional GCN (DGCN).

Math per layer (matches PyG GCNConv with self-loops + symmetric norm):
  hf = Ahat_f @ (h @ Wf) + bf      (messages src->dst)
  hb = Ahat_b @ (h @ Wb) + bb      (messages dst->src)
  h  = relu(concat(hf, hb))        (no relu on last layer)

Device strategy (8 NeuronCores, SPMD):
  - nodes padded to N'=8*NPC, core c owns node rows [c*NPC, (c+1)*NPC)
  - edges partitioned by destination core; per-edge metadata (gather index +
    dest slot) is host-precomputed graph preprocessing
  - symmetric norm folded structurally: hs = (h @ W) * dinv[src] at the
    transform epilogue (ACT per-partition scale), out = dinv[dst] * sum + b
    at the aggregation epilogue; the selector matrix is a pure one-hot
  - transform: h kept channel-major [256, NPC] so h tiles are direct matmul
    lhsT operands; produces the node-major gather table hs
  - layer 0: full x replicated on every core -> no communication
  - layers 1,2: sharded transform + AllGather of hs (4 quarter sub-AGs per
    direction so the next layer can start as quarters arrive)
  - aggregation: dma_gather 128-edge chunks from hs (node-major rows), build
    the one-hot S01[e, b, v] = (iota[v]==dstloc[e,b]) for a whole gather call
    in ONE batched DVE tensor_tensor (stride-0 broadcast APs), then
    matmul-accumulate on PE: PSUM[ch,v] += G[e,ch]^T @ S[e,v].  Per
    dest-block of 128 nodes there are exactly KW chunks per source-window
    (host pads edge groups), so the SPMD program is identical on every core.
  - final layer runs the matmul the other way round (PSUM[v,ch] += S^T @ G)
    to emit node-major fp32 output directly.
"""

import numpy as np

P = 128          # partition size / dest-block size / feature width
N_CORES = 8


def _bf16():
    import ml_dtypes
    return ml_dtypes.bfloat16


class Cfg:
    def __init__(self, n_nodes, hid=128, npc=12800, sb=6, feat="bf16"):
        assert npc % 512 == 0
        self.n_nodes = n_nodes
        self.hid = hid
        self.npc = npc                      # nodes per core (padded)
        self.np_total = npc * N_CORES
        assert self.np_total >= n_nodes
        self.qsz = npc // 4                 # rows per quarter (per core)
        self.nbq = self.qsz // P            # blocks per quarter
        self.bpc = npc // P                 # blocks per core
        self.agrows = 8 * self.qsz          # rows of each AG window buffer
        assert self.agrows <= 32767         # int16 gather indices
        self.sb = sb                        # superblock: live PSUM tiles
        self.feat = feat                    # "bf16" or "f32"

    @property
    def runs(self):
        return [list(range(i, min(i + self.sb, self.bpc)))
                for i in range(0, self.bpc, self.sb)]


# ----------------------------------------------------------------------------
# host-side graph preprocessing
# ----------------------------------------------------------------------------

def _group_edges(s, d, cfg):
    """Sort one core's incoming edges by (dest block, source window)."""
    npc, qsz = cfg.npc, cfg.qsz
    lb = (d % npc) // P
    qs = (s % npc) // qsz
    key = lb * 4 + qs
    order = np.argsort(key, kind="stable")
    return order, key[order]


def _scan_kw(s, d, cfg):
    kw = 1
    core_of = d // cfg.npc
    for c in range(N_CORES):
        m = core_of == c
        _, key = _group_edges(s[m], d[m], cfg)
        counts = np.bincount(key, minlength=cfg.bpc * 4)
        kw = max(kw, int(np.ceil(counts.max() / P)))
    return kw


def _build_direction(s, d, cfg, kw, dt_np):
    """Per-core padded edge metadata for one message direction.

    Per core:
      idx [128, idx_cols] int16 : gather row indices, per-call 16-row wrapped
                                  and replicated across the 8 Q7 cores
      dst [128, nchunks] feat   : dest slot within block (-1 for pad edges)
    Order: for run in superblock-runs: for q in windows: for b in run: chunks.
    """
    npc, qsz, bpc = cfg.npc, cfg.qsz, cfg.bpc
    core_of = d // npc
    slots = kw * P
    out = []
    for c in range(N_CORES):
        m = core_of == c
        es, ed = s[m], d[m]
        order, key = _group_edges(es, ed, cfg)
        es, ed = es[order], ed[order]
        counts = np.bincount(key, minlength=bpc * 4)
        assert counts.max() <= slots
        starts = np.zeros(bpc * 4, np.int64)
        starts[1:] = np.cumsum(counts)[:-1]
        rank = np.arange(len(key)) - starts[key]
        flatpos = key.astype(np.int64) * slots + rank
        idx_pad = np.zeros(bpc * 4 * slots, np.int16)
        dst_pad = np.full(bpc * 4 * slots, -1.0, np.float32)
        idx_pad[flatpos] = ((es // npc) * qsz + (es % npc) % qsz).astype(np.int16)
        dst_pad[flatpos] = (ed % P).astype(np.float32)
        idx_pad = idx_pad.reshape(bpc, 4, kw, P)
        dst_pad = dst_pad.reshape(bpc, 4, kw, P)
        idx_cols, dst_cols = [], []
        for run in cfg.runs:
            for q in range(4):
                flat = idx_pad[run[0]:run[-1] + 1, q].reshape(-1)
                idx_cols.append(np.tile(flat.reshape(-1, 16).T, (8, 1)))
                dst_cols.append(dst_pad[run[0]:run[-1] + 1, q].reshape(-1, P).T)
        out.append(dict(
            idx=np.ascontiguousarray(np.concatenate(idx_cols, axis=1)),
            dst=np.ascontiguousarray(
                np.concatenate(dst_cols, axis=1)).astype(dt_np),
        ))
    return out


def host_prep(inputs, cfg):
    x = np.asarray(inputs["x"], np.float32)
    ei = np.asarray(inputs["edge_index"], np.int64)
    src, dst = ei[0], ei[1]
    n = cfg.n_nodes
    sl = np.arange(n, dtype=np.int64)
    pad1 = np.concatenate([np.ones(n), np.zeros(cfg.np_total - n)])

    deg_f = np.bincount(dst, minlength=cfg.np_total) + pad1
    deg_b = np.bincount(src, minlength=cfg.np_total) + pad1
    with np.errstate(divide="ignore"):
        dinv_f = np.where(deg_f > 0, 1.0 / np.sqrt(deg_f), 0.0)
        dinv_f = dinv_f.astype(np.float32)
        dinv_b = np.where(deg_b > 0, 1.0 / np.sqrt(deg_b), 0.0)
        dinv_b = dinv_b.astype(np.float32)

    dt_np = np.float32 if cfg.feat == "f32" else _bf16()

    sf = np.concatenate([src, sl])
    df = np.concatenate([dst, sl])
    kw = max(_scan_kw(sf, df, cfg), _scan_kw(df, sf, cfg))
    meta_f = _build_direction(sf, df, cfg, kw, dt_np)
    meta_b = _build_direction(df, sf, cfg, kw, dt_np)

    xp = np.zeros((cfg.np_total, cfg.hid), np.float32)
    xp[:n] = x
    x_cm = np.ascontiguousarray(xp.T).astype(dt_np)

    wslots = []
    for lname in ("0", "1", "2"):
        for dname in ("f", "b"):
            W = np.asarray(inputs[f"W{dname}{lname}"], np.float32)
            if W.shape[0] == cfg.hid:
                wslots.append(W)
            else:
                wslots.append(W[:cfg.hid])
                wslots.append(W[cfg.hid:])
    w_all = np.concatenate(wslots, axis=1).astype(dt_np)

    bias_all = np.zeros((128, 6), np.float32)
    for li, lname in enumerate(("0", "1", "2")):
        for di, dname in enumerate(("f", "b")):
            bias_all[:, li * 2 + di] = np.asarray(
                inputs[f"b{dname}{lname}"], np.float32)
    bias2 = np.zeros((128, 256), np.float32)
    bias2[:, 0:128] = np.asarray(inputs["bf2"], np.float32)[None, :]
    bias2[:, 128:256] = np.asarray(inputs["bb2"], np.float32)[None, :]

    iota_t = np.tile(np.arange(P, dtype=np.float32)[None, :],
                     (P, 1)).astype(dt_np)

    # per-partition dinv columns: col g holds dinv[g*128:(g+1)*128]
    dinv_cols = {
        "f": np.ascontiguousarray(
            dinv_f.reshape(8 * cfg.bpc, P).T),       # [128, 8*bpc] f32
        "b": np.ascontiguousarray(
            dinv_b.reshape(8 * cfg.bpc, P).T),
    }
    in_maps = []
    for c in range(N_CORES):
        # broadcast dinv over partitions for the free-dim epilogue scaling
        dbc = {
            dn: np.ascontiguousarray(np.tile(
                dv[c * cfg.npc:(c + 1) * cfg.npc][None, :],
                (P, 1))).astype(dt_np)
            for dn, dv in (("f", dinv_f), ("b", dinv_b))
        }
        in_maps.append(dict(
            x_cm=x_cm,
            w_all=w_all,
            bias_all=bias_all,
            bias2=bias2,
            iota_t=iota_t,
            dinv_all_f=dinv_cols["f"],
            dinv_all_b=dinv_cols["b"],
            dinv_loc_f=np.ascontiguousarray(
                dinv_f[c * cfg.npc:(c + 1) * cfg.npc].reshape(cfg.bpc, P).T),
            dinv_loc_b=np.ascontiguousarray(
                dinv_b[c * cfg.npc:(c + 1) * cfg.npc].reshape(cfg.bpc, P).T),
            dinv_bc_f=dbc["f"],
            dinv_bc_b=dbc["b"],
            meta_idx_f=meta_f[c]["idx"],
            meta_dst_f=meta_f[c]["dst"],
            meta_idx_b=meta_b[c]["idx"],
            meta_dst_b=meta_b[c]["dst"],
        ))
    return in_maps, kw


# ----------------------------------------------------------------------------
# device program
# ----------------------------------------------------------------------------

def build_program(cfg, kw):
    import bass_rust
    import concourse.bass as bass
    import concourse.bacc as bacc
    import concourse.tile as tile
    import concourse.mybir as mybir
    from concourse._compat import get_trn_type
    from contextlib import ExitStack

    dt = mybir.dt
    dt_feat = dt.float32 if cfg.feat == "f32" else dt.bfloat16
    AF = mybir.ActivationFunctionType
    ALU = mybir.AluOpType

    npc, qsz, nbq, bpc, sb = cfg.npc, cfg.qsz, cfg.nbq, cfg.bpc, cfg.sb
    runs = cfg.runs
    nchunks = bpc * 4 * kw
    idx_cols_total = nchunks * P // 16

    def bcast_last(ap, n):
        return bass_rust.AP(ap.tensor, ap.offset, list(ap.ap) + [[0, n]])

    def bcast_mid(ap, n):
        a = list(ap.ap)
        return bass_rust.AP(ap.tensor, ap.offset, [a[0], [0, n], a[1]])

    nc = bacc.Bacc(get_trn_type() or "TRN2", target_bir_lowering=False,
                   debug=False, num_swdge_queues=4)

    x_cm = nc.dram_tensor("x_cm", [P, cfg.np_total], dt_feat,
                          kind="ExternalInput")
    w_all = nc.dram_tensor("w_all", [P, 10 * P], dt_feat, kind="ExternalInput")
    bias_all = nc.dram_tensor("bias_all", [P, 6], dt.float32,
                              kind="ExternalInput")
    bias2 = nc.dram_tensor("bias2", [P, 256], dt.float32, kind="ExternalInput")
    iota_in = nc.dram_tensor("iota_t", [P, P], dt_feat, kind="ExternalInput")
    dinv_all = {d: nc.dram_tensor(f"dinv_all_{d}", [P, 8 * bpc], dt.float32,
                                  kind="ExternalInput") for d in ("f", "b")}
    dinv_bc = {d: nc.dram_tensor(f"dinv_bc_{d}", [P, npc], dt_feat,
                                 kind="ExternalInput") for d in ("f", "b")}
    meta = {}
    for dname in ("f", "b"):
        meta[dname] = dict(
            idx=nc.dram_tensor(f"meta_idx_{dname}", [128, idx_cols_total],
                               dt.int16, kind="ExternalInput"),
            dst=nc.dram_tensor(f"meta_dst_{dname}", [P, nchunks], dt_feat,
                               kind="ExternalInput"),
        )
    out_t = nc.dram_tensor("out", [npc, 2 * P], dt.float32,
                           kind="ExternalOutput")

    agbuf = {d: [nc.dram_tensor(f"ag_{d}{q}", [cfg.agrows, P], dt_feat,
                                addr_space="Shared")
                 for q in range(4)] for d in ("f", "b")}
    hs_loc = {d: [nc.dram_tensor(f"hs_{d}{q}", [qsz, P], dt_feat)
                  for q in range(4)] for d in ("f", "b")}
    h_q = [nc.dram_tensor(f"h_{q}", [2 * P, qsz], dt_feat) for q in range(4)]

    wslot = {}
    col = 0
    for li in range(3):
        for dname in ("f", "b"):
            for h in range(1 if li == 0 else 2):
                wslot[(li, dname, h)] = col
                col += P

    with tile.TileContext(nc) as tc, ExitStack() as ctx:
        cpool = ctx.enter_context(tc.tile_pool(name="consts", bufs=1))
        xhpool = ctx.enter_context(tc.tile_pool(name="xh", bufs=2))
        gpool = ctx.enter_context(tc.tile_pool(name="gather", bufs=5))
        ipool = ctx.enter_context(tc.tile_pool(name="idx", bufs=5))
        mpool = ctx.enter_context(tc.tile_pool(name="meta", bufs=5))
        spool = ctx.enter_context(tc.tile_pool(name="sel", bufs=4))
        dpool = ctx.enter_context(tc.tile_pool(name="dinvq", bufs=3))
        tpool = ctx.enter_context(tc.tile_pool(name="tmp", bufs=4))
        stpool = ctx.enter_context(tc.tile_pool(name="stage", bufs=3))
        st2pool = ctx.enter_context(tc.tile_pool(name="stage2", bufs=2))
        pa = ctx.enter_context(tc.tile_pool(name="psum_a", bufs=sb,
                                            space="PSUM"))
        pt = ctx.enter_context(tc.tile_pool(name="psum_t", bufs=2,
                                            space="PSUM"))

        iota_s = cpool.tile([P, P], dt_feat, tag="iota")
        nc.sync.dma_start(out=iota_s[:, :], in_=iota_in[:, :])
        w_s = cpool.tile([P, 10 * P], dt_feat, tag="wall")
        nc.sync.dma_start(out=w_s[:, :], in_=w_all[:, :])
        bias_s = cpool.tile([P, 6], dt.float32, tag="bias")
        nc.sync.dma_start(out=bias_s[:, :], in_=bias_all[:, :])
        bias2_s = cpool.tile([P, 256], dt.float32, tag="bias2")
        nc.sync.dma_start(out=bias2_s[:, :], in_=bias2[:, :])
        dinv_all_s = {}
        for dname in ("f", "b"):
            t = cpool.tile([P, 8 * bpc], dt.float32, tag=f"dinva_{dname}",
                           name=f"dinva_{dname}")
            nc.sync.dma_start(out=t[:, :], in_=dinv_all[dname][:, :])
            dinv_all_s[dname] = t

        def wap(li, dname, h):
            c0 = wslot[(li, dname, h)]
            return w_s[:, c0:c0 + P]

        def dinv_col(dname, g):
            """per-partition dinv for global block g (f32 [128,1])."""
            return dinv_all_s[dname][:, g:g + 1]

        # ---- layer 0 transform: full x -> agbuf directly -------------------
        def l0_transform():
            for q in range(4):
                for r in range(N_CORES):
                    c0 = r * npc + q * qsz
                    xt = xhpool.tile([P, qsz], dt_feat, tag="ht")
                    nc.sync.dma_start(out=xt[:, :], in_=x_cm[:, c0:c0 + qsz])
                    for dname in ("f", "b"):
                        stg = stpool.tile([P, qsz], dt_feat, tag="stg")
                        for j in range(nbq):
                            g = r * bpc + q * nbq + j
                            ps = pt.tile([P, P], dt.float32, tag="pt")
                            nc.tensor.matmul(ps[:, :],
                                             lhsT=xt[:, j * P:(j + 1) * P],
                                             rhs=wap(0, dname, 0),
                                             start=True, stop=True)
                            nc.scalar.activation(stg[:, j * P:(j + 1) * P],
                                                 ps[:, :], AF.Copy,
                                                 scale=dinv_col(dname, g))
                        nc.sync.dma_start(
                            out=agbuf[dname][q][r * qsz:(r + 1) * qsz, :]
                            .rearrange("(j p) c -> p j c", p=P),
                            in_=stg[:, :].rearrange("p (j c) -> p j c", c=P))

        def aggregate(li, core_rank_unused=None):
            last = li == 2
            for dname in ("f", "b"):
                di = 0 if dname == "f" else 1
                md = meta[dname]
                chunk0 = 0
                icol0 = 0
                stg_q = {}
                dinvq = {}
                blocks_done = {q: 0 for q in range(4)}
                for run in runs:
                    nb = len(run)
                    psums = {b: pa.tile([P, P], dt.float32, tag="pa",
                                        name=f"pa_{li}{dname}{b}")
                             for b in run}
                    for q in range(4):
                        L = nb * kw * P
                        icols = L // 16
                        it = ipool.tile([128, sb * kw * P // 16], dt.int16,
                                        tag="idx")
                        nc.sync.dma_start(
                            out=it[:, 0:icols],
                            in_=md["idx"][:, icol0:icol0 + icols])
                        gt = gpool.tile([P, sb * kw, P], dt_feat, tag="g")
                        nc.gpsimd.dma_gather(
                            out_ap=gt[:, 0:nb * kw, :],
                            in_ap=agbuf[dname][q][:, :],
                            idxs_ap=it[:, 0:icols],
                            num_idxs=L,
                            num_idxs_reg=L,
                            elem_size=P,
                            single_packet=False,
                            queue_num=q,
                        )
                        dtile = mpool.tile([P, sb * kw], dt_feat, tag="dst")
                        nc.sync.dma_start(
                            out=dtile[:, 0:nb * kw],
                            in_=md["dst"][:, chunk0:chunk0 + nb * kw])
                        st = spool.tile([P, sb * kw, P], dt_feat, tag="sel")
                        nc.vector.tensor_tensor(
                            st[:, 0:nb * kw, :],
                            bcast_mid(iota_s[:, :], nb * kw),
                            bcast_last(dtile[:, 0:nb * kw], P),
                            ALU.is_equal)
                        for bi, b in enumerate(run):
                            for k in range(kw):
                                cc = bi * kw + k
                                first = (q == 0 and k == 0)
                                lastmm = (q == 3 and k == kw - 1)
                                if last:
                                    nc.tensor.matmul(
                                        psums[b][:, :], lhsT=st[:, cc, :],
                                        rhs=gt[:, cc, :],
                                        start=first, stop=lastmm)
                                else:
                                    nc.tensor.matmul(
                                        psums[b][:, :], lhsT=gt[:, cc, :],
                                        rhs=st[:, cc, :],
                                        start=first, stop=lastmm)
                        chunk0 += nb * kw
                        icol0 += icols
                    for b in run:
                        qb = b // nbq
                        j = b % nbq
                        if qb not in stg_q:
                            if last:
                                stg_q[qb] = st2pool.tile(
                                    [P, qsz], dt.float32, tag="st2",
                                    name=f"st2_{dname}{qb}")
                            else:
                                stg_q[qb] = stpool.tile(
                                    [P, qsz], dt_feat, tag="stg",
                                    name=f"stgh_{li}{dname}{qb}")
                        if last:
                            # out[v,ch] = dinv_d[v]*psum + bias2[ch]
                            nc.vector.scalar_tensor_tensor(
                                out=stg_q[qb][:, j * P:(j + 1) * P],
                                in0=psums[b][:, :],
                                scalar=dinv_loc_s[dname][:, b:b + 1],
                                in1=bias2_s[:, di * P:(di + 1) * P],
                                op0=ALU.mult, op1=ALU.add)
                        else:
                            if qb not in dinvq:
                                dq = dpool.tile([P, qsz], dt_feat,
                                                tag="dinvq",
                                                name=f"dq_{li}{dname}{qb}")
                                nc.sync.dma_start(
                                    out=dq[:, :],
                                    in_=dinv_bc[dname][:, qb * qsz:
                                                       (qb + 1) * qsz])
                                dinvq[qb] = dq
                            tmp = tpool.tile([P, P], dt.float32, tag="tmp")
                            nc.vector.tensor_tensor(
                                tmp[:, :], psums[b][:, :],
                                dinvq[qb][:, j * P:(j + 1) * P], ALU.mult)
                            bcol = li * 2 + di
                            nc.scalar.activation(
                                stg_q[qb][:, j * P:(j + 1) * P],
                                tmp[:, :], AF.Relu,
                                bias=bias_s[:, bcol:bcol + 1])
                        blocks_done[qb] += 1
                        if blocks_done[qb] == nbq:
                            if last:
                                nc.sync.dma_start(
                                    out=out_t[qb * qsz:(qb + 1) * qsz,
                                              di * P:(di + 1) * P]
                                    .rearrange("(j p) c -> p j c", p=P),
                                    in_=stg_q[qb][:, :]
                                    .rearrange("p (j c) -> p j c", c=P))
                            else:
                                nc.sync.dma_start(
                                    out=h_q[qb][di * P:(di + 1) * P, :],
                                    in_=stg_q[qb][:, :])
                            del stg_q[qb]
                            dinvq.pop(qb, None)

        def transform(li, core_rank):
            for dname in ("f", "b"):
                for q in range(4):
                    h0 = xhpool.tile([P, qsz], dt_feat, tag="ht")
                    h1 = xhpool.tile([P, qsz], dt_feat, tag="ht")
                    nc.sync.dma_start(out=h0[:, :], in_=h_q[q][0:P, :])
                    nc.sync.dma_start(out=h1[:, :], in_=h_q[q][P:2 * P, :])
                    stg = stpool.tile([P, qsz], dt_feat, tag="stg")
                    for j in range(nbq):
                        ps = pt.tile([P, P], dt.float32, tag="pt")
                        nc.tensor.matmul(ps[:, :],
                                         lhsT=h0[:, j * P:(j + 1) * P],
                                         rhs=wap(li, dname, 0),
                                         start=True, stop=False)
                        nc.tensor.matmul(ps[:, :],
                                         lhsT=h1[:, j * P:(j + 1) * P],
                                         rhs=wap(li, dname, 1),
                                         start=False, stop=True)
                        nc.scalar.activation(
                            stg[:, j * P:(j + 1) * P], ps[:, :], AF.Copy,
                            scale=dinv_loc_col(dname, q * nbq + j))
                    nc.sync.dma_start(
                        out=hs_loc[dname][q][:, :]
                        .rearrange("(j p) c -> p j c", p=P),
                        in_=stg[:, :].rearrange("p (j c) -> p j c", c=P))
                    nc.gpsimd.collective_compute(
                        "AllGather",
                        mybir.AluOpType.bypass,
                        replica_groups=[list(range(N_CORES))],
                        ins=[hs_loc[dname][q][:, :].opt()],
                        outs=[agbuf[dname][q][:, :].opt()],
                    )

        def dinv_loc_col(dname, lb):
            # local block lb of this core = global block rank*bpc+lb; but the
            # SPMD program is shared, so index via the per-core broadcast
            # input instead: dinv_bc[dname][:, lb*P:(lb+1)*P] column j holds
            # dinv of node lb*P+j, identical down the partition dim.  For a
            # per-partition [128,1] scale we need dinv values ALONG
            # partitions, which dinv_bc cannot provide; use dinv_loc below.
            return dinv_loc_s[dname][:, lb:lb + 1]

        # per-core local per-partition dinv columns: [128, bpc]
        dinv_loc = {d: nc.dram_tensor(f"dinv_loc_{d}", [P, bpc], dt.float32,
                                      kind="ExternalInput")
                    for d in ("f", "b")}
        dinv_loc_s = {}
        for dname in ("f", "b"):
            t = cpool.tile([P, bpc], dt.float32, tag=f"dinvl_{dname}",
                           name=f"dinvl_{dname}")
            nc.sync.dma_start(out=t[:, :], in_=dinv_loc[dname][:, :])
            dinv_loc_s[dname] = t

        with nc.named_scope("L0T"):
            l0_transform()
        with nc.named_scope("agg0"):
            aggregate(0)
        with nc.named_scope("T1"):
            transform(1, None)
        with nc.named_scope("agg1"):
            aggregate(1)
        with nc.named_scope("T2"):
            transform(2, None)
        with nc.named_scope("agg2"):
            aggregate(2)

    nc.compile()
    return nc


# ----------------------------------------------------------------------------
# entry point
# ----------------------------------------------------------------------------

def run(inputs, cfg, trace=False):
    from concourse.bass_utils import run_bass_kernel_spmd
    in_maps, kw = host_prep(inputs, cfg)
    nc = build_program(cfg, kw)
    res = run_bass_kernel_spmd(nc, in_maps, list(range(N_CORES)), trace=trace)
    outs = [res.results[c]["out"] for c in range(N_CORES)]
    full = np.concatenate(outs, axis=0)[:cfg.n_nodes]
    return np.asarray(full, np.float32), res


def kernel(**inputs):
    cfg = Cfg(n_nodes=inputs["x"].shape[0])
    out, _ = run(inputs, cfg)
    return out



# revision 5
# speedup vs baseline: 2.1894x; 1.4924x over previous
"""Bass/Trainium2 kernel for a 3-layer bidirectional GCN (DGCN).

Math per layer (matches PyG GCNConv with self-loops + symmetric norm):
  hf = Ahat_f @ (h @ Wf) + bf      (messages src->dst)
  hb = Ahat_b @ (h @ Wb) + bb      (messages dst->src)
  h  = relu(concat(hf, hb))        (no relu on last layer)

Device strategy (8 NeuronCores, SPMD):
  - nodes padded to N'=8*NPC, core c owns node rows [c*NPC, (c+1)*NPC)
  - edges partitioned by destination core; per-edge metadata (gather index +
    dest slot) is host-precomputed graph preprocessing
  - symmetric norm folded structurally: hs = (h @ W) * dinv[src] at the
    transform epilogue, out = dinv[dst] * sum + b at the aggregation epilogue
  - self-loops are NOT general gathered edges: layer 0 carries them as one
    extra EXACT chunk per dest block (idx = own rows, dst = iota - pure
    per-core data, SPMD-safe); layers 1/2 add them with one identity matmul
    per block from the node-major local hs_loc rows.  This drops the
    per-(block,window) chunk budget kw by one.
  - layer 0 transform is replicated (x is a free full-replica input), writes
    agbuf directly; layers 1/2 transform locally and AllGather hs in 4
    quarter sub-AGs per direction, INJECTED into the previous layer's
    aggregation run loop at quarter-completion points so AGs and transforms
    overlap the gather stream.
  - aggregation: dma_gather 128-edge chunks from hs (node-major rows), build
    the one-hot S01[e, b, v] = (iota[v]==dstloc[e,b]) for a whole gather call
    in ONE batched DVE tensor_tensor (stride-0 broadcast APs), then
    matmul-accumulate on PE: PSUM[ch,v] += G[e,ch]^T @ S[e,v].
  - the 4 per-window gathers of each run go to SWDGE queues 0-3 so
    descriptor generation spreads over all four Q7 core pairs; f/b runs
    alternate to keep the pipe full.
  - meta (idx+dst) is loaded in one batched DMA per (run, direction) issued
    on the ACT HWDGE ring so it cannot be blocked by stores on the Sync ring.
  - epilogues store per-block (PSUM -> [P,P] tile -> DRAM), final layer in
    node-major fp32 straight to the output.
"""

import numpy as np

P = 128          # partition size / dest-block size / feature width
N_CORES = 8


def _bf16():
    import ml_dtypes
    return ml_dtypes.bfloat16


class Cfg:
    def __init__(self, n_nodes, hid=128, npc=12800, sb=6, feat="bf16"):
        assert npc % 512 == 0
        self.n_nodes = n_nodes
        self.hid = hid
        self.npc = npc                      # nodes per core (padded)
        self.np_total = npc * N_CORES
        assert self.np_total >= n_nodes
        self.qsz = npc // 4                 # rows per quarter (per core)
        self.nbq = self.qsz // P            # blocks per quarter
        self.bpc = npc // P                 # blocks per core
        self.agrows = 8 * self.qsz          # rows of each AG window buffer
        assert self.agrows <= 32767         # int16 gather indices
        self.sb = sb                        # superblock: live PSUM tiles
        self.feat = feat                    # "bf16" or "f32"

    @property
    def runs(self):
        return [list(range(i, min(i + self.sb, self.bpc)))
                for i in range(0, self.bpc, self.sb)]


def _extra(cfg, b, q, sl_chunk):
    """Self-loop chunks appended to block b's window-q group."""
    return 1 if (sl_chunk and q == b // cfg.nbq) else 0


# ----------------------------------------------------------------------------
# host-side graph preprocessing
# ----------------------------------------------------------------------------

def _group_edges(s, d, cfg):
    """Sort one core's incoming edges by (dest block, source window)."""
    npc, qsz = cfg.npc, cfg.qsz
    lb = (d % npc) // P
    qs = (s % npc) // qsz
    key = lb * 4 + qs
    order = np.argsort(key, kind="stable")
    return order, key[order]


def _scan_kw(s, d, cfg):
    kw = 1
    core_of = d // cfg.npc
    for c in range(N_CORES):
        m = core_of == c
        _, key = _group_edges(s[m], d[m], cfg)
        counts = np.bincount(key, minlength=cfg.bpc * 4)
        kw = max(kw, int(np.ceil(counts.max() / P)))
    return kw


def _build_direction(s, d, cfg, kw, dt_np, sl_chunk):
    """Per-core padded edge metadata for one message direction.

    Per core:
      idx [128, idx_cols] int16 : gather row indices, per-call 16-row wrapped
                                  and replicated across the 8 Q7 cores
      dst [128, nchunks] feat   : dest slot within block (-1 for pad edges)
    Order: for run in superblock-runs: for q in windows: for b in run: chunks.
    If sl_chunk, block b's own-window group gets one extra chunk carrying the
    128 self-loop edges (idx = the block's own agbuf rows, dst = 0..127).
    """
    npc, qsz, bpc, nbq = cfg.npc, cfg.qsz, cfg.bpc, cfg.nbq
    core_of = d // npc
    slots = kw * P
    out = []
    for c in range(N_CORES):
        m = core_of == c
        es, ed = s[m], d[m]
        order, key = _group_edges(es, ed, cfg)
        es, ed = es[order], ed[order]
        counts = np.bincount(key, minlength=bpc * 4)
        assert counts.max() <= slots
        starts = np.zeros(bpc * 4, np.int64)
        starts[1:] = np.cumsum(counts)[:-1]
        rank = np.arange(len(key)) - starts[key]
        flatpos = key.astype(np.int64) * slots + rank
        idx_pad = np.zeros(bpc * 4 * slots, np.int16)
        dst_pad = np.full(bpc * 4 * slots, -1.0, np.float32)
        idx_pad[flatpos] = ((es // npc) * qsz + (es % npc) % qsz).astype(np.int16)
        dst_pad[flatpos] = (ed % P).astype(np.float32)
        idx_pad = idx_pad.reshape(bpc, 4, kw, P)
        dst_pad = dst_pad.reshape(bpc, 4, kw, P)
        idx_cols, dst_cols = [], []
        for run in cfg.runs:
            for q in range(4):
                iparts, dparts = [], []
                for b in run:
                    iparts.append(idx_pad[b, q].reshape(-1))
                    dparts.append(dst_pad[b, q].reshape(-1))
                    if _extra(cfg, b, q, sl_chunk):
                        # self-loop chunk: node (b*P+i) of this core; its
                        # agbuf row is c*qsz + (local row within quarter)
                        j = b % nbq
                        rows = c * qsz + j * P + np.arange(P)
                        iparts.append(rows.astype(np.int16))
                        dparts.append(np.arange(P, dtype=np.float32))
                flat = np.concatenate(iparts)
                dflat = np.concatenate(dparts)
                idx_cols.append(np.tile(flat.reshape(-1, 16).T, (8, 1)))
                dst_cols.append(dflat.reshape(-1, P).T)
        out.append(dict(
            idx=np.ascontiguousarray(np.concatenate(idx_cols, axis=1)),
            dst=np.ascontiguousarray(
                np.concatenate(dst_cols, axis=1)).astype(dt_np),
        ))
    return out


def host_prep(inputs, cfg):
    x = np.asarray(inputs["x"], np.float32)
    ei = np.asarray(inputs["edge_index"], np.int64)
    src, dst = ei[0], ei[1]
    n = cfg.n_nodes
    pad1 = np.concatenate([np.ones(n), np.zeros(cfg.np_total - n)])

    deg_f = np.bincount(dst, minlength=cfg.np_total) + pad1
    deg_b = np.bincount(src, minlength=cfg.np_total) + pad1
    with np.errstate(divide="ignore"):
        dinv_f = np.where(deg_f > 0, 1.0 / np.sqrt(deg_f), 0.0)
        dinv_f = dinv_f.astype(np.float32)
        dinv_b = np.where(deg_b > 0, 1.0 / np.sqrt(deg_b), 0.0)
        dinv_b = dinv_b.astype(np.float32)

    dt_np = np.float32 if cfg.feat == "f32" else _bf16()

    # self-loops are handled densely on-device; only real edges get grouped
    kw = max(_scan_kw(src, dst, cfg), _scan_kw(dst, src, cfg))
    meta0_f = _build_direction(src, dst, cfg, kw, dt_np, True)
    meta0_b = _build_direction(dst, src, cfg, kw, dt_np, True)
    meta_f = _build_direction(src, dst, cfg, kw, dt_np, False)
    meta_b = _build_direction(dst, src, cfg, kw, dt_np, False)

    xp = np.zeros((cfg.np_total, cfg.hid), np.float32)
    xp[:n] = x
    x_cm = np.ascontiguousarray(xp.T).astype(dt_np)

    wslots = []
    for lname in ("0", "1", "2"):
        for dname in ("f", "b"):
            W = np.asarray(inputs[f"W{dname}{lname}"], np.float32)
            if W.shape[0] == cfg.hid:
                wslots.append(W)
            else:
                wslots.append(W[:cfg.hid])
                wslots.append(W[cfg.hid:])
    w_all = np.concatenate(wslots, axis=1).astype(dt_np)

    bias_all = np.zeros((128, 6), np.float32)
    for li, lname in enumerate(("0", "1", "2")):
        for di, dname in enumerate(("f", "b")):
            bias_all[:, li * 2 + di] = np.asarray(
                inputs[f"b{dname}{lname}"], np.float32)
    bias2 = np.zeros((128, 256), np.float32)
    bias2[:, 0:128] = np.asarray(inputs["bf2"], np.float32)[None, :]
    bias2[:, 128:256] = np.asarray(inputs["bb2"], np.float32)[None, :]

    iota_t = np.tile(np.arange(P, dtype=np.float32)[None, :],
                     (P, 1)).astype(dt_np)
    ident = np.eye(P, dtype=np.float32).astype(dt_np)

    dinv_cols = {
        "f": np.ascontiguousarray(dinv_f.reshape(8 * cfg.bpc, P).T),
        "b": np.ascontiguousarray(dinv_b.reshape(8 * cfg.bpc, P).T),
    }
    in_maps = []
    for c in range(N_CORES):
        # broadcast dinv over partitions for the free-dim epilogue scaling
        dbc = {
            dn: np.ascontiguousarray(np.tile(
                dv[c * cfg.npc:(c + 1) * cfg.npc][None, :],
                (P, 1))).astype(dt_np)
            for dn, dv in (("f", dinv_f), ("b", dinv_b))
        }
        in_maps.append(dict(
            x_cm=x_cm,
            w_all=w_all,
            bias_all=bias_all,
            bias2=bias2,
            iota_t=iota_t,
            ident=ident,
            dinv_all_f=dinv_cols["f"],
            dinv_all_b=dinv_cols["b"],
            dinv_loc_f=np.ascontiguousarray(
                dinv_f[c * cfg.npc:(c + 1) * cfg.npc].reshape(cfg.bpc, P).T),
            dinv_loc_b=np.ascontiguousarray(
                dinv_b[c * cfg.npc:(c + 1) * cfg.npc].reshape(cfg.bpc, P).T),
            dinv_bc_f=dbc["f"],
            dinv_bc_b=dbc["b"],
            meta0_idx_f=meta0_f[c]["idx"],
            meta0_dst_f=meta0_f[c]["dst"],
            meta0_idx_b=meta0_b[c]["idx"],
            meta0_dst_b=meta0_b[c]["dst"],
            meta_idx_f=meta_f[c]["idx"],
            meta_dst_f=meta_f[c]["dst"],
            meta_idx_b=meta_b[c]["idx"],
            meta_dst_b=meta_b[c]["dst"],
        ))
    return in_maps, kw


# ----------------------------------------------------------------------------
# device program
# ----------------------------------------------------------------------------

def build_program(cfg, kw):
    import bass_rust
    import concourse.bass as bass
    import concourse.bacc as bacc
    import concourse.tile as tile
    import concourse.mybir as mybir
    from concourse._compat import get_trn_type
    from contextlib import ExitStack

    dt = mybir.dt
    dt_feat = dt.float32 if cfg.feat == "f32" else dt.bfloat16
    AF = mybir.ActivationFunctionType
    ALU = mybir.AluOpType

    npc, qsz, nbq, bpc, sb = cfg.npc, cfg.qsz, cfg.nbq, cfg.bpc, cfg.sb
    runs = cfg.runs

    def nchunks_dir(sl_chunk):
        return bpc * 4 * kw + (bpc if sl_chunk else 0)

    def bcast_last(ap, n):
        return bass_rust.AP(ap.tensor, ap.offset, list(ap.ap) + [[0, n]])

    def bcast_mid(ap, n):
        a = list(ap.ap)
        return bass_rust.AP(ap.tensor, ap.offset, [a[0], [0, n], a[1]])

    nc = bacc.Bacc(get_trn_type() or "TRN2", target_bir_lowering=False,
                   debug=False, num_swdge_queues=4)

    x_cm = nc.dram_tensor("x_cm", [P, cfg.np_total], dt_feat,
                          kind="ExternalInput")
    w_all = nc.dram_tensor("w_all", [P, 10 * P], dt_feat, kind="ExternalInput")
    bias_all = nc.dram_tensor("bias_all", [P, 6], dt.float32,
                              kind="ExternalInput")
    bias2 = nc.dram_tensor("bias2", [P, 256], dt.float32, kind="ExternalInput")
    iota_in = nc.dram_tensor("iota_t", [P, P], dt_feat, kind="ExternalInput")
    ident_in = nc.dram_tensor("ident", [P, P], dt_feat, kind="ExternalInput")
    dinv_all = {d: nc.dram_tensor(f"dinv_all_{d}", [P, 8 * bpc], dt.float32,
                                  kind="ExternalInput") for d in ("f", "b")}
    dinv_loc = {d: nc.dram_tensor(f"dinv_loc_{d}", [P, bpc], dt.float32,
                                  kind="ExternalInput") for d in ("f", "b")}
    dinv_bc = {d: nc.dram_tensor(f"dinv_bc_{d}", [P, npc], dt_feat,
                                 kind="ExternalInput") for d in ("f", "b")}
    meta = {}
    for li0 in (True, False):
        pre = "meta0" if li0 else "meta"
        nch = nchunks_dir(li0)
        for dname in ("f", "b"):
            meta[(li0, dname)] = dict(
                idx=nc.dram_tensor(f"{pre}_idx_{dname}", [128, nch * P // 16],
                                   dt.int16, kind="ExternalInput"),
                dst=nc.dram_tensor(f"{pre}_dst_{dname}", [P, nch], dt_feat,
                                   kind="ExternalInput"),
            )
    out_t = nc.dram_tensor("out", [npc, 2 * P], dt.float32,
                           kind="ExternalOutput")

    # double-buffered by layer parity: layer li gathers from set li%2 while
    # the fused AllGather for layer li+1 writes set (li+1)%2
    agbuf = {(par, d): [nc.dram_tensor(f"ag{par}_{d}{q}",
                                       [cfg.agrows, P], dt_feat,
                                       addr_space="Shared")
                        for q in range(4)]
             for par in (0, 1) for d in ("f", "b")}
    hs_loc = {d: [nc.dram_tensor(f"hs_{d}{q}", [qsz, P], dt_feat)
                  for q in range(4)] for d in ("f", "b")}
    h_q = [nc.dram_tensor(f"h_{q}", [2 * P, qsz], dt_feat) for q in range(4)]

    wslot = {}
    col = 0
    for li in range(3):
        for dname in ("f", "b"):
            for h in range(1 if li == 0 else 2):
                wslot[(li, dname, h)] = col
                col += P

    # max chunks in one gather call (worst case: all run blocks in one quarter)
    mxch = sb * (kw + 1)

    with tile.TileContext(nc) as tc, ExitStack() as ctx:
        cpool = ctx.enter_context(tc.tile_pool(name="consts", bufs=1))
        xhpool = ctx.enter_context(tc.tile_pool(name="xh", bufs=2))
        gpool = ctx.enter_context(tc.tile_pool(name="gather", bufs=5))
        ipool = ctx.enter_context(tc.tile_pool(name="idx", bufs=3))
        mpool = ctx.enter_context(tc.tile_pool(name="meta", bufs=3))
        spool = ctx.enter_context(tc.tile_pool(name="sel", bufs=4))
        dpool = ctx.enter_context(tc.tile_pool(name="dinvq", bufs=4))
        tpool = ctx.enter_context(tc.tile_pool(name="tmp", bufs=6))
        opool = ctx.enter_context(tc.tile_pool(name="outb", bufs=6))
        hrpool = ctx.enter_context(tc.tile_pool(name="hsrow", bufs=12))
        stpool = ctx.enter_context(tc.tile_pool(name="stage", bufs=3))
        pa = ctx.enter_context(tc.tile_pool(name="psum_a", bufs=sb,
                                            space="PSUM"))
        pt = ctx.enter_context(tc.tile_pool(name="psum_t", bufs=2,
                                            space="PSUM"))

        iota_s = cpool.tile([P, P], dt_feat, tag="iota")
        nc.sync.dma_start(out=iota_s[:, :], in_=iota_in[:, :])
        ident_s = cpool.tile([P, P], dt_feat, tag="ident")
        nc.sync.dma_start(out=ident_s[:, :], in_=ident_in[:, :])
        w_s = cpool.tile([P, 10 * P], dt_feat, tag="wall")
        nc.sync.dma_start(out=w_s[:, :], in_=w_all[:, :])
        bias_s = cpool.tile([P, 6], dt.float32, tag="bias")
        nc.sync.dma_start(out=bias_s[:, :], in_=bias_all[:, :])
        bias2_s = cpool.tile([P, 256], dt.float32, tag="bias2")
        nc.sync.dma_start(out=bias2_s[:, :], in_=bias2[:, :])
        dinv_all_s = {}
        dinv_loc_s = {}
        for dname in ("f", "b"):
            t = cpool.tile([P, 8 * bpc], dt.float32, tag=f"dinva_{dname}",
                           name=f"dinva_{dname}")
            nc.sync.dma_start(out=t[:, :], in_=dinv_all[dname][:, :])
            dinv_all_s[dname] = t
            t = cpool.tile([P, bpc], dt.float32, tag=f"dinvl_{dname}",
                           name=f"dinvl_{dname}")
            nc.sync.dma_start(out=t[:, :], in_=dinv_loc[dname][:, :])
            dinv_loc_s[dname] = t

        def wap(li, dname, h):
            c0 = wslot[(li, dname, h)]
            return w_s[:, c0:c0 + P]

        # ---- layer 0 transform: replicated full x -> agbuf directly --------
        def l0_transform():
            for q in range(4):
                for r in range(N_CORES):
                    c0 = r * npc + q * qsz
                    xt = xhpool.tile([P, qsz], dt_feat, tag="ht")
                    nc.scalar.dma_start(out=xt[:, :], in_=x_cm[:, c0:c0 + qsz])
                    for di, dname in enumerate(("f", "b")):
                        stg = stpool.tile([P, qsz], dt_feat, tag="stg")
                        for j in range(nbq):
                            g = r * bpc + q * nbq + j
                            ps = pt.tile([P, P], dt.float32, tag="pt")
                            nc.tensor.matmul(ps[:, :],
                                             lhsT=xt[:, j * P:(j + 1) * P],
                                             rhs=wap(0, dname, 0),
                                             start=True, stop=True)
                            dv = dinv_all_s[dname][:, g:g + 1]
                            # split the PSUM->SBUF scaled copies over ACT and
                            # DVE so neither serializes the startup phase
                            if (r + q) % 2 == di:
                                nc.scalar.activation(
                                    stg[:, j * P:(j + 1) * P], ps[:, :],
                                    AF.Copy, scale=dv)
                            else:
                                nc.vector.tensor_scalar(
                                    stg[:, j * P:(j + 1) * P], ps[:, :],
                                    dv, None, ALU.mult)
                        nc.sync.dma_start(
                            out=agbuf[(0, dname)][q][r * qsz:(r + 1) * qsz, :]
                            .rearrange("(j p) c -> p j c", p=P),
                            in_=stg[:, :].rearrange("p (j c) -> p j c", c=P))

        # ---- transform of one quarter for layer li (1 or 2) + AllGather ----
        def transform_quarter(li, dname, q):
            h0 = xhpool.tile([P, qsz], dt_feat, tag="ht")
            h1 = xhpool.tile([P, qsz], dt_feat, tag="ht")
            nc.sync.dma_start(out=h0[:, :], in_=h_q[q][0:P, :])
            nc.sync.dma_start(out=h1[:, :], in_=h_q[q][P:2 * P, :])
            stg = stpool.tile([P, qsz], dt_feat, tag="stg")
            for j in range(nbq):
                ps = pt.tile([P, P], dt.float32, tag="pt")
                nc.tensor.matmul(ps[:, :], lhsT=h0[:, j * P:(j + 1) * P],
                                 rhs=wap(li, dname, 0), start=True, stop=False)
                nc.tensor.matmul(ps[:, :], lhsT=h1[:, j * P:(j + 1) * P],
                                 rhs=wap(li, dname, 1), start=False, stop=True)
                nc.scalar.activation(
                    stg[:, j * P:(j + 1) * P], ps[:, :], AF.Copy,
                    scale=dinv_loc_s[dname][:, q * nbq + j:q * nbq + j + 1])
            nc.sync.dma_start(
                out=hs_loc[dname][q][:, :].rearrange("(j p) c -> p j c", p=P),
                in_=stg[:, :].rearrange("p (j c) -> p j c", c=P))
            nc.gpsimd.collective_compute(
                "AllGather",
                mybir.AluOpType.bypass,
                replica_groups=[list(range(N_CORES))],
                ins=[hs_loc[dname][q][:, :].opt()],
                outs=[agbuf[(li % 2, dname)][q][:, :].opt()],
            )

        # ---- one aggregation run (one direction, sb dest blocks) -----------
        # state[(li,dname)] = dict(chunk0, icol0) advancing through the meta
        def run_body(li, dname, run, state):
            last = li == 2
            sl_chunk = li == 0
            di = 0 if dname == "f" else 1
            md = meta[(sl_chunk, dname)]
            st8 = state[(li, dname)]
            nb = len(run)
            # per-window chunk counts for this run
            wch = [nb * kw + sum(_extra(cfg, b, q, sl_chunk) for b in run)
                   for q in range(4)]
            tch = sum(wch)
            psums = {b: pa.tile([P, P], dt.float32, tag="pa",
                                name=f"pa_{li}{dname}{b}")
                     for b in run}
            # batched meta load for the whole run (ACT HWDGE ring)
            it = ipool.tile([128, 4 * mxch * P // 16], dt.int16, tag="idx")
            nc.scalar.dma_start(
                out=it[:, 0:tch * P // 16],
                in_=md["idx"][:, st8["icol0"]:st8["icol0"] + tch * P // 16])
            dtile = mpool.tile([P, 4 * mxch], dt_feat, tag="dst")
            nc.scalar.dma_start(
                out=dtile[:, 0:tch],
                in_=md["dst"][:, st8["chunk0"]:st8["chunk0"] + tch])
            hsrows = {}
            if not sl_chunk:
                for b in run:
                    hr = hrpool.tile([P, P], dt_feat, tag="hsrow")
                    nc.scalar.dma_start(
                        out=hr[:, :],
                        in_=hs_loc[dname][b // nbq]
                        [(b % nbq) * P:((b % nbq) + 1) * P, :])
                    hsrows[b] = hr
            coff = 0
            for q in range(4):
                L = wch[q] * P
                icols = L // 16
                gt = gpool.tile([P, mxch, P], dt_feat, tag="g")
                nc.gpsimd.dma_gather(
                    out_ap=gt[:, 0:wch[q], :],
                    in_ap=agbuf[(li % 2, dname)][q][:, :],
                    idxs_ap=it[:, coff * P // 16:coff * P // 16 + icols],
                    num_idxs=L,
                    num_idxs_reg=L,
                    elem_size=P,
                    single_packet=False,
                    queue_num=q,
                )
                st = spool.tile([P, mxch, P], dt_feat, tag="sel")
                nc.vector.tensor_tensor(
                    st[:, 0:wch[q], :],
                    bcast_mid(iota_s[:, :], wch[q]),
                    bcast_last(dtile[:, coff:coff + wch[q]], P),
                    ALU.is_equal)
                cc = 0
                for b in run:
                    nch = kw + _extra(cfg, b, q, sl_chunk)
                    for k in range(nch):
                        first = (q == 0 and k == 0)
                        lastmm = (sl_chunk and q == 3 and k == nch - 1)
                        if last:
                            nc.tensor.matmul(
                                psums[b][:, :], lhsT=st[:, cc, :],
                                rhs=gt[:, cc, :], start=first, stop=lastmm)
                        else:
                            nc.tensor.matmul(
                                psums[b][:, :], lhsT=gt[:, cc, :],
                                rhs=st[:, cc, :], start=first, stop=lastmm)
                        cc += 1
                coff += wch[q]
            st8["chunk0"] += tch
            st8["icol0"] += tch * P // 16
            if not sl_chunk:
                for b in run:
                    # dense self-loop add closes the accumulation chain
                    if last:
                        nc.tensor.matmul(
                            psums[b][:, :], lhsT=ident_s[:, :],
                            rhs=hsrows[b][:, :], start=False, stop=True)
                    else:
                        nc.tensor.matmul(
                            psums[b][:, :], lhsT=hsrows[b][:, :],
                            rhs=ident_s[:, :], start=False, stop=True)
            for b in run:
                qb = b // nbq
                j = b % nbq
                if last:
                    # out[v,ch] = dinv_d[v]*psum + bias2[ch]
                    ob = opool.tile([P, P], dt.float32, tag="outb")
                    nc.vector.scalar_tensor_tensor(
                        out=ob[:, :], in0=psums[b][:, :],
                        scalar=dinv_loc_s[dname][:, b:b + 1],
                        in1=bias2_s[:, di * P:(di + 1) * P],
                        op0=ALU.mult, op1=ALU.add)
                    nc.sync.dma_start(
                        out=out_t[b * P:(b + 1) * P, di * P:(di + 1) * P],
                        in_=ob[:, :])
                else:
                    dq = state["dinvq"].get((dname, qb))
                    if dq is None:
                        dq = dpool.tile([P, qsz], dt_feat, tag="dinvq",
                                        name=f"dq_{li}{dname}{qb}")
                        nc.sync.dma_start(
                            out=dq[:, :],
                            in_=dinv_bc[dname][:, qb * qsz:(qb + 1) * qsz])
                        state["dinvq"][(dname, qb)] = dq
                    tmp = tpool.tile([P, P], dt.float32, tag="tmp")
                    nc.vector.tensor_tensor(
                        tmp[:, :], psums[b][:, :],
                        dq[:, j * P:(j + 1) * P], ALU.mult)
                    bcol = li * 2 + di
                    ob = opool.tile([P, P], dt_feat, tag="outb")
                    nc.scalar.activation(ob[:, :], tmp[:, :], AF.Relu,
                                         bias=bias_s[:, bcol:bcol + 1])
                    nc.sync.dma_start(
                        out=h_q[qb][di * P:(di + 1) * P, j * P:(j + 1) * P],
                        in_=ob[:, :])

        # ---- fused per-layer pipeline --------------------------------------
        def layer_body(li):
            state = {(li, "f"): dict(chunk0=0, icol0=0),
                     (li, "b"): dict(chunk0=0, icol0=0),
                     "dinvq": {}}
            # quarter q of both directions completes during run index rq[q]
            rq = {}
            for q in range(4):
                lastb = (q + 1) * nbq - 1
                for ri, run in enumerate(runs):
                    if lastb in run:
                        rq[ri] = q
            for ri, run in enumerate(runs):
                for dname in ("f", "b"):
                    run_body(li, dname, run, state)
                if li < 2 and ri in rq:
                    q = rq[ri]
                    for dname in ("f", "b"):
                        transform_quarter(li + 1, dname, q)

        with nc.named_scope("L0T"):
            l0_transform()
        with nc.named_scope("agg0"):
            layer_body(0)
        with nc.named_scope("agg1"):
            layer_body(1)
        with nc.named_scope("agg2"):
            layer_body(2)

    nc.compile()
    return nc


# ----------------------------------------------------------------------------
# entry point
# ----------------------------------------------------------------------------

def run(inputs, cfg, trace=False):
    from concourse.bass_utils import run_bass_kernel_spmd
    in_maps, kw = host_prep(inputs, cfg)
    nc = build_program(cfg, kw)
    res = run_bass_kernel_spmd(nc, in_maps, list(range(N_CORES)), trace=trace)
    outs = [res.results[c]["out"] for c in range(N_CORES)]
    full = np.concatenate(outs, axis=0)[:cfg.n_nodes]
    return np.asarray(full, np.float32), res


def kernel(**inputs):
    cfg = Cfg(n_nodes=inputs["x"].shape[0])
    out, _ = run(inputs, cfg)
    return out
